# revision 7
# baseline (speedup 1.0000x reference)
"""Additive attention (d2l-style) on 8 TRN2 NeuronCores via Bass/Tile.

Problem shapes: B=16, Q=128, K=512, H=128, DQ=DK=DV=128 (all f32).
Sharding: data-parallel over batch, 2 batch elements per core, params
replicated. No collectives needed.

Per-core per-batch pipeline:
  PE  : qpT = W_q^T @ q^T  (h,q);  kpT = W_k^T @ k^T  (h,k)   [f32 matmuls]
  DVE : g[:, j*K:(j+1)*K] = kpT + qpT[:, q]  (broadcast add, f32, 2x mode)
  ACT : t = tanh(g)  -> bf16, big 4096-wide tiles (overhead amortized)
  PE  : scores[q, :] += w_v^T @ t_q  (bf16 matvec, accumulated into a PSUM
        tile pre-initialized with the -1e6 key mask via a broadcast matmul)
  ACT : expo = exp(scores), accum_out = row sums   (no max-subtraction:
        |scores| <= sum|w_v| ~ 10, well within f32 exp range; masked keys
        get exp(-1e6) = 0 exactly)
  PE  : attn^T via transposes; out = attn^T.T @ values  (f32)
  DVE : out *= 1/rowsum  -> DMA out
"""

import numpy as np

B, Q, K, H, D = 16, 128, 512, 128, 128
NCORES = 8
B_LOC = B // NCORES
MASK_VALUE = -1000000.0
QG = 8  # queries per ACT tanh tile


def _build_graph():
    from concourse import bacc, tile, mybir, masks

    f32 = mybir.dt.float32
    bf16 = mybir.dt.bfloat16
    AF = mybir.ActivationFunctionType

    nc = bacc.Bacc("TRN2", target_bir_lowering=False, num_devices=NCORES)

    qT = nc.dram_tensor("qT", [B_LOC, D, Q], f32, kind="ExternalInput").ap()
    kT = nc.dram_tensor("kT", [B_LOC, D, K], f32, kind="ExternalInput").ap()
    vals = nc.dram_tensor("vals", [B_LOC, K, D], f32, kind="ExternalInput").ap()
    mask = nc.dram_tensor("mask", [B_LOC, 1, K], f32, kind="ExternalInput").ap()
    Wq = nc.dram_tensor("Wq", [D, H], f32, kind="ExternalInput").ap()
    Wk = nc.dram_tensor("Wk", [D, H], f32, kind="ExternalInput").ap()
    wv = nc.dram_tensor("wv", [H, 1], f32, kind="ExternalInput").ap()
    out = nc.dram_tensor("out", [B_LOC, Q, D], f32, kind="ExternalOutput").ap()

    with tile.TileContext(nc) as tc:
        with (
            tc.tile_pool(name="const", bufs=1) as const,
            tc.tile_pool(name="inp", bufs=2) as inp,
            tc.tile_pool(name="proj_ps", bufs=1, space="PSUM") as proj_ps,
            tc.tile_pool(name="proj_sb", bufs=2) as proj_sb,
            tc.tile_pool(name="g_pool", bufs=3) as g_pool,
            tc.tile_pool(name="t_pool", bufs=3) as t_pool,
            tc.tile_pool(name="sc_ps", bufs=2, space="PSUM") as sc_ps,
            tc.tile_pool(name="soft_sb", bufs=2) as soft_sb,
            tc.tile_pool(name="at_ps", bufs=1, space="PSUM") as at_ps,
            tc.tile_pool(name="at_sb", bufs=3) as at_sb,
            tc.tile_pool(name="av_ps", bufs=1, space="PSUM") as av_ps,
            tc.tile_pool(name="out_sb", bufs=2) as out_sb,
        ):
            wq_t = const.tile([D, H], f32)
            nc.sync.dma_start(wq_t[:], Wq[:])
            wk_t = const.tile([D, H], f32)
            nc.sync.dma_start(wk_t[:], Wk[:])
            wv_f32 = const.tile([H, 1], f32)
            nc.sync.dma_start(wv_f32[:], wv[:])
            # wv_diag[:, j*32:(j+1)*32] is a (H, 32) stationary tile holding
            # w_v in column j, zeros elsewhere: a matvec with it writes
            # w_v . t into row j of a 32-row PSUM band (PE outputs must
            # start at 32-aligned partitions) and zeros into the others.
            wv_diag = const.tile([H, 32 * 32], bf16)
            nc.vector.memset(wv_diag[:], 0.0)
            for j in range(32):
                nc.vector.tensor_copy(wv_diag[:, j * 32 + j : j * 32 + j + 1], wv_f32[:])
            ones_row = const.tile([1, Q], bf16)
            nc.vector.memset(ones_row[:], 1.0)
            ident = const.tile([128, 128], f32)
            masks.make_identity(nc, ident[:])

            for b in range(B_LOC):
                qT_sb = inp.tile([D, Q], f32, tag="qT_sb")
                nc.sync.dma_start(qT_sb[:], qT[b])
                kT_sb = inp.tile([D, K], f32, tag="kT_sb")
                nc.sync.dma_start(kT_sb[:], kT[b])
                # values (K, D) -> SBUF (128, 4*128): partition = k % 128,
                # free = (k_tile, v)
                vals_sb = inp.tile([128, (K // 128) * D], f32, tag="vals_sb")
                for kt in range(K // 128):
                    nc.sync.dma_start(
                        vals_sb[:, kt * D : (kt + 1) * D],
                        vals[b, kt * 128 : (kt + 1) * 128, :],
                    )
                mask_sb = inp.tile([1, K], f32, tag="mask_sb")
                nc.sync.dma_start(mask_sb[:], mask[b])
                mask_bf = inp.tile([1, K], bf16, tag="mask_bf")
                nc.vector.tensor_copy(mask_bf[:], mask_sb[:])

                # projections: qpT (h, q), kpT (h, k), both f32
                qpT_ps = proj_ps.tile([H, Q], f32, tag="qpT_ps")
                nc.tensor.matmul(qpT_ps[:], wq_t[:], qT_sb[:], start=True, stop=True)
                qpT = proj_sb.tile([H, Q], f32, tag="qpT")
                nc.vector.tensor_copy(qpT[:], qpT_ps[:])
                kpT_ps = proj_ps.tile([H, K], f32, tag="kpT_ps")
                nc.tensor.matmul(kpT_ps[:], wk_t[:], kT_sb[:], start=True, stop=True)
                kpT = proj_sb.tile([H, K], f32, tag="kpT")
                nc.vector.tensor_copy(kpT[:], kpT_ps[:])

                # scores in two (64, K) PSUM tiles (PE output base partitions
                # are limited to {0,32,64}; 96 is unencodable), each
                # pre-initialized with the key mask (broadcast over q via a
                # rank-1 matmul), then accumulated by per-q matvecs over h
                sc_half = [
                    sc_ps.tile([Q // 2, K], f32, tag=f"sc{h_}", name=f"sc{h_}")
                    for h_ in range(2)
                ]
                for h_ in range(2):
                    nc.tensor.matmul(
                        sc_half[h_][:], ones_row[:, : Q // 2], mask_bf[:],
                        start=True, stop=False, skip_group_check=True,
                    )
                for g8 in range(Q // QG):
                    g = g_pool.tile([H, QG * K], f32, tag="g")
                    for j in range(QG):
                        q = g8 * QG + j
                        nc.vector.tensor_scalar_add(
                            g[:, j * K : (j + 1) * K], kpT[:], qpT[:, q : q + 1]
                        )
                    tt = t_pool.tile([H, QG * K], bf16, tag="tt")
                    nc.scalar.activation(tt[:], g[:], AF.Tanh)
                    for j in range(QG):
                        q = g8 * QG + j
                        half, qh = q // 64, q % 64
                        band, lane = (qh // 32) * 32, qh % 32
                        nc.tensor.matmul(
                            sc_half[half][band : band + 32, :],
                            wv_diag[:, lane * 32 : (lane + 1) * 32],
                            tt[:, j * K : (j + 1) * K],
                            start=False,
                            stop=(qh == 63),
                            skip_group_check=True,
                        )

                # softmax over k (free axis); no max subtraction needed
                expo = soft_sb.tile([Q, K], f32, tag="expo")
                sumexp = soft_sb.tile([Q, 1], f32, tag="sumexp")
                for h_ in range(2):
                    nc.scalar.activation(
                        expo[h_ * 64 : (h_ + 1) * 64, :],
                        sc_half[h_][:],
                        AF.Exp,
                        accum_out=sumexp[h_ * 64 : (h_ + 1) * 64, :],
                    )
                rec = soft_sb.tile([Q, 1], f32, tag="rec")
                nc.vector.reciprocal(rec[:], sumexp[:])

                # out = (attn^T)^T @ values, then scale rows by 1/sumexp
                av = av_ps.tile([Q, D], f32, tag="av")
                for kt in range(K // 128):
                    aT_ps = at_ps.tile([128, Q], f32, tag="aT_ps")
                    nc.tensor.transpose(
                        aT_ps[:], expo[:, kt * 128 : (kt + 1) * 128], ident[:]
                    )
                    aT = at_sb.tile([128, Q], f32, tag="aT")
                    nc.vector.tensor_copy(aT[:], aT_ps[:])
                    nc.tensor.matmul(
                        av[:],
                        aT[:],
                        vals_sb[:, kt * D : (kt + 1) * D],
                        start=(kt == 0),
                        stop=(kt == K // 128 - 1),
                    )
                ot = out_sb.tile([Q, D], f32, tag="ot")
                nc.vector.tensor_scalar_mul(ot[:], av[:], rec[:])
                nc.sync.dma_start(out[b], ot[:])

    nc.finalize()
    return nc


_NC_CACHE = None


def kernel(queries, keys, values, valid_lens, W_q, W_k, w_v):
    global _NC_CACHE
    from concourse.bass_utils import run_bass_kernel_spmd

    queries = np.asarray(queries, dtype=np.float32)
    keys = np.asarray(keys, dtype=np.float32)
    values = np.asarray(values, dtype=np.float32)
    valid_lens = np.asarray(valid_lens)
    W_q = np.asarray(W_q, dtype=np.float32)
    W_k = np.asarray(W_k, dtype=np.float32)
    w_v = np.asarray(w_v, dtype=np.float32)

    queriesT = np.ascontiguousarray(queries.transpose(0, 2, 1))  # (B, D, Q)
    keysT = np.ascontiguousarray(keys.transpose(0, 2, 1))  # (B, D, K)
    mask_add = np.where(
        np.arange(K)[None, :] < valid_lens[:, None], 0.0, MASK_VALUE
    ).astype(np.float32)[:, None, :]  # (B, 1, K)
    wv2 = np.ascontiguousarray(w_v.reshape(H, 1))

    if _NC_CACHE is None:
        _NC_CACHE = _build_graph()
    nc = _NC_CACHE

    in_maps = []
    for c in range(NCORES):
        bs = [c * B_LOC + j for j in range(B_LOC)]
        in_maps.append(
            {
                "qT": np.ascontiguousarray(queriesT[bs]),
                "kT": np.ascontiguousarray(keysT[bs]),
                "vals": np.ascontiguousarray(values[bs]),
                "mask": np.ascontiguousarray(mask_add[bs]),
                "Wq": W_q,
                "Wk": W_k,
                "wv": wv2,
            }
        )

    res = run_bass_kernel_spmd(nc, in_maps, core_ids=list(range(NCORES)))

    out = np.empty((B, Q, D), dtype=np.float32)
    for c in range(NCORES):
        for j in range(B_LOC):
            out[c * B_LOC + j] = res.results[c]["out"][j]
    return out


# revision 11
# speedup vs baseline: 1.1820x; 1.1820x over previous
"""Additive attention (d2l-style) on 8 TRN2 NeuronCores via Bass/Tile.

Problem shapes: B=16, Q=128, K=512, H=128, DQ=DK=DV=128 (all f32).
Sharding: data-parallel over batch, 2 batch elements per core, params
replicated. No collectives needed.

Per-core per-batch pipeline:
  PE  : qpT = W_q^T @ q^T  (h,q);  kpT = W_k^T @ k^T  (h,k)   [f32 matmuls]
  DVE : g[:, j*K:(j+1)*K] = kpT + qpT[:, q]  (broadcast add, f32, 2x mode)
  ACT : t = tanh(g)  -> bf16, big 4096-wide tiles (overhead amortized)
  PE  : scores[q, :] += w_v^T @ t_q  (bf16 matvec, accumulated into a PSUM
        tile pre-initialized with the -1e6 key mask via a broadcast matmul)
  ACT : expo = exp(scores), accum_out = row sums   (no max-subtraction:
        |scores| <= sum|w_v| ~ 10, well within f32 exp range; masked keys
        get exp(-1e6) = 0 exactly)
  PE  : attn^T via transposes; out = attn^T.T @ values  (f32)
  DVE : out *= 1/rowsum  -> DMA out
"""

import numpy as np

B, Q, K, H, D = 16, 128, 512, 128, 128
NCORES = 8
B_LOC = B // NCORES
MASK_VALUE = -1000000.0
QG = 16  # queries per ACT tanh tile

# q-order such that runs of 4 consecutive matvecs share one stationary
# w_v column (lane = q % 32): for each lane, the 4 bands {0,32,64,96}
Q_ORDER = [band + lane for lane in range(32) for band in (0, 32, 64, 96)]


def _build_graph():
    from concourse import bacc, tile, mybir, masks

    f32 = mybir.dt.float32
    bf16 = mybir.dt.bfloat16
    AF = mybir.ActivationFunctionType

    nc = bacc.Bacc("TRN2", target_bir_lowering=False, num_devices=NCORES)

    qT = nc.dram_tensor("qT", [B_LOC, D, Q], f32, kind="ExternalInput").ap()
    kT = nc.dram_tensor("kT", [B_LOC, D, K], f32, kind="ExternalInput").ap()
    vals = nc.dram_tensor("vals", [B_LOC, K, D], f32, kind="ExternalInput").ap()
    mask = nc.dram_tensor("mask", [B_LOC, 1, K], f32, kind="ExternalInput").ap()
    Wq = nc.dram_tensor("Wq", [D, H], f32, kind="ExternalInput").ap()
    Wk = nc.dram_tensor("Wk", [D, H], f32, kind="ExternalInput").ap()
    wv = nc.dram_tensor("wv", [H, 1], f32, kind="ExternalInput").ap()
    out = nc.dram_tensor("out", [B_LOC, Q, D], f32, kind="ExternalOutput").ap()

    with tile.TileContext(nc) as tc:
        with (
            tc.tile_pool(name="const", bufs=1) as const,
            tc.tile_pool(name="inp", bufs=2) as inp,
            tc.tile_pool(name="proj_ps", bufs=1, space="PSUM") as proj_ps,
            tc.tile_pool(name="proj_sb", bufs=2) as proj_sb,
            tc.tile_pool(name="g_pool", bufs=3) as g_pool,
            tc.tile_pool(name="t_pool", bufs=3) as t_pool,
            tc.tile_pool(name="sc_ps", bufs=2, space="PSUM") as sc_ps,
            tc.tile_pool(name="soft_sb", bufs=2) as soft_sb,
            tc.tile_pool(name="at_ps", bufs=1, space="PSUM") as at_ps,
            tc.tile_pool(name="at_sb", bufs=3) as at_sb,
            tc.tile_pool(name="av_ps", bufs=1, space="PSUM") as av_ps,
            tc.tile_pool(name="out_sb", bufs=2) as out_sb,
        ):
            wq_t = const.tile([D, H], f32)
            nc.sync.dma_start(wq_t[:], Wq[:])
            wk_t = const.tile([D, H], f32)
            nc.sync.dma_start(wk_t[:], Wk[:])
            wv_f32 = const.tile([H, 1], f32)
            nc.sync.dma_start(wv_f32[:], wv[:])
            # wv_diag[:, j*32:(j+1)*32] is a (H, 32) stationary tile holding
            # w_v in column j, zeros elsewhere: a matvec with it writes
            # w_v . t into row j of a 32-row PSUM band (PE outputs must
            # start at 32-aligned partitions) and zeros into the others.
            wv_diag = const.tile([H, 32 * 32], bf16)
            nc.vector.memset(wv_diag[:], 0.0)
            for j in range(32):
                nc.vector.tensor_copy(wv_diag[:, j * 32 + j : j * 32 + j + 1], wv_f32[:])
            ones_row = const.tile([1, Q], bf16)
            nc.vector.memset(ones_row[:], 1.0)
            ident = const.tile([128, 128], f32)
            masks.make_identity(nc, ident[:])

            for b in range(B_LOC):
                qT_sb = inp.tile([D, Q], f32, tag="qT_sb")
                nc.sync.dma_start(qT_sb[:], qT[b])
                kT_sb = inp.tile([D, K], f32, tag="kT_sb")
                nc.sync.dma_start(kT_sb[:], kT[b])
                # values (K, D) -> SBUF (128, 4*128): partition = k % 128,
                # free = (k_tile, v)
                vals_sb = inp.tile([128, (K // 128) * D], f32, tag="vals_sb")
                for kt in range(K // 128):
                    nc.sync.dma_start(
                        vals_sb[:, kt * D : (kt + 1) * D],
                        vals[b, kt * 128 : (kt + 1) * 128, :],
                    )
                mask_sb = inp.tile([1, K], f32, tag="mask_sb")
                nc.sync.dma_start(mask_sb[:], mask[b])
                mask_bf = inp.tile([1, K], bf16, tag="mask_bf")
                nc.vector.tensor_copy(mask_bf[:], mask_sb[:])

                # projections: qpT (h, q), kpT (h, k), both f32
                qpT_ps = proj_ps.tile([H, Q], f32, tag="qpT_ps")
                nc.tensor.matmul(qpT_ps[:], wq_t[:], qT_sb[:], start=True, stop=True)
                qpT = proj_sb.tile([H, Q], f32, tag="qpT")
                nc.vector.tensor_copy(qpT[:], qpT_ps[:])
                kpT_ps = proj_ps.tile([H, K], f32, tag="kpT_ps")
                nc.tensor.matmul(kpT_ps[:], wk_t[:], kT_sb[:], start=True, stop=True)
                kpT = proj_sb.tile([H, K], bf16, tag="kpT")
                nc.vector.tensor_copy(kpT[:], kpT_ps[:])

                # scores in two (64, K) PSUM tiles (PE output base partitions
                # are limited to {0,32,64}; 96 is unencodable), each
                # pre-initialized with the key mask (broadcast over q via a
                # rank-1 matmul), then accumulated by per-q matvecs over h
                sc_half = [
                    sc_ps.tile([Q // 2, K], f32, tag=f"sc{h_}", name=f"sc{h_}")
                    for h_ in range(2)
                ]
                for h_ in range(2):
                    nc.tensor.matmul(
                        sc_half[h_][:], ones_row[:, : Q // 2], mask_bf[:],
                        start=True, stop=False, skip_group_check=True,
                    )
                n_matvecs_left = [64, 64]  # per sc half, to place stop flags
                for g8 in range(Q // QG):
                    g = g_pool.tile([H, QG * K], bf16, tag="g")
                    for j in range(QG):
                        q = Q_ORDER[g8 * QG + j]
                        nc.vector.tensor_scalar_add(
                            g[:, j * K : (j + 1) * K], kpT[:], qpT[:, q : q + 1]
                        )
                    tt = t_pool.tile([H, QG * K], bf16, tag="tt")
                    nc.scalar.activation(tt[:], g[:], AF.Tanh)
                    for j in range(QG):
                        q = Q_ORDER[g8 * QG + j]
                        half, qh = q // 64, q % 64
                        band, lane = (qh // 32) * 32, qh % 32
                        n_matvecs_left[half] -= 1
                        nc.tensor.matmul(
                            sc_half[half][band : band + 32, :],
                            wv_diag[:, lane * 32 : (lane + 1) * 32],
                            tt[:, j * K : (j + 1) * K],
                            start=False,
                            stop=(n_matvecs_left[half] == 0),
                            skip_group_check=True,
                        )

                # softmax over k (free axis); no max subtraction needed
                expo = soft_sb.tile([Q, K], f32, tag="expo")
                sumexp = soft_sb.tile([Q, 1], f32, tag="sumexp")
                for h_ in range(2):
                    nc.scalar.activation(
                        expo[h_ * 64 : (h_ + 1) * 64, :],
                        sc_half[h_][:],
                        AF.Exp,
                        accum_out=sumexp[h_ * 64 : (h_ + 1) * 64, :],
                    )
                rec = soft_sb.tile([Q, 1], f32, tag="rec")
                nc.vector.reciprocal(rec[:], sumexp[:])

                # out = (attn^T)^T @ values, then scale rows by 1/sumexp
                av = av_ps.tile([Q, D], f32, tag="av")
                for kt in range(K // 128):
                    aT_ps = at_ps.tile([128, Q], f32, tag="aT_ps")
                    nc.tensor.transpose(
                        aT_ps[:], expo[:, kt * 128 : (kt + 1) * 128], ident[:]
                    )
                    aT = at_sb.tile([128, Q], f32, tag="aT")
                    nc.vector.tensor_copy(aT[:], aT_ps[:])
                    nc.tensor.matmul(
                        av[:],
                        aT[:],
                        vals_sb[:, kt * D : (kt + 1) * D],
                        start=(kt == 0),
                        stop=(kt == K // 128 - 1),
                    )
                ot = out_sb.tile([Q, D], f32, tag="ot")
                nc.vector.tensor_scalar_mul(ot[:], av[:], rec[:])
                nc.sync.dma_start(out[b], ot[:])

    nc.finalize()
    return nc


_NC_CACHE = None


def kernel(queries, keys, values, valid_lens, W_q, W_k, w_v):
    global _NC_CACHE
    from concourse.bass_utils import run_bass_kernel_spmd

    queries = np.asarray(queries, dtype=np.float32)
    keys = np.asarray(keys, dtype=np.float32)
    values = np.asarray(values, dtype=np.float32)
    valid_lens = np.asarray(valid_lens)
    W_q = np.asarray(W_q, dtype=np.float32)
    W_k = np.asarray(W_k, dtype=np.float32)
    w_v = np.asarray(w_v, dtype=np.float32)

    queriesT = np.ascontiguousarray(queries.transpose(0, 2, 1))  # (B, D, Q)
    keysT = np.ascontiguousarray(keys.transpose(0, 2, 1))  # (B, D, K)
    mask_add = np.where(
        np.arange(K)[None, :] < valid_lens[:, None], 0.0, MASK_VALUE
    ).astype(np.float32)[:, None, :]  # (B, 1, K)
    wv2 = np.ascontiguousarray(w_v.reshape(H, 1))

    if _NC_CACHE is None:
        _NC_CACHE = _build_graph()
    nc = _NC_CACHE

    in_maps = []
    for c in range(NCORES):
        bs = [c * B_LOC + j for j in range(B_LOC)]
        in_maps.append(
            {
                "qT": np.ascontiguousarray(queriesT[bs]),
                "kT": np.ascontiguousarray(keysT[bs]),
                "vals": np.ascontiguousarray(values[bs]),
                "mask": np.ascontiguousarray(mask_add[bs]),
                "Wq": W_q,
                "Wk": W_k,
                "wv": wv2,
            }
        )

    res = run_bass_kernel_spmd(nc, in_maps, core_ids=list(range(NCORES)))

    out = np.empty((B, Q, D), dtype=np.float32)
    for c in range(NCORES):
        for j in range(B_LOC):
            out[c * B_LOC + j] = res.results[c]["out"][j]
    return out


# revision 14
# speedup vs baseline: 1.4955x; 1.2652x over previous
"""Additive attention (d2l-style) on 8 TRN2 NeuronCores via Bass/Tile.

Problem shapes: B=16, Q=128, K=512, H=128, DQ=DK=DV=128 (all f32).

Sharding: every core runs the SAME graph over ALL 16 batch elements but
only a 16-query slice of each (core c owns q in [16c, 16c+16)). This
keeps SPMD work perfectly uniform while letting the graph be specialized
to the actual valid_lens: per batch b, only FD_b = round_up(vl_b, 2)
key columns are ever touched (the softmax weight of the rest is exactly
0), which cuts the dominant tanh-features work by ~2x in expectation.
The graph is rebuilt (and recompiled, ~2s) per distinct valid_lens.

Per-core per-batch-section pipeline:
  PE  : qpT = W_q^T @ q_cT (h,16) f32;  kpT = W_k^T @ k_bT (h,FD_b) bf16
  DVE : g[:, j*FD:(j+1)*FD] = kpT + qpT[:, j]  (bf16 broadcast add)
  ACT : t = tanh(g) -> bf16, one (128, 16*FD_b) tile per section
  PE  : sc[j, :FD_b] += w_v . t_j  (bf16 matvec into a (16, FD_b) PSUM
        tile pre-initialized with the key mask via a rank-1 matmul)
  ACT : expo = exp(sc) -> bf16, accum_out = row sums (no max-subtraction:
        |scores| <= sum|w_v| ~ 10; masked tail of the last 128-key tile
        is zeroed explicitly)
  PE  : attn^T via transposes; av = attn^T.T @ values (bf16)
  DVE : out = av * (1/rowsum) -> f32 -> DMA out
"""

import numpy as np

B, Q, K, H, D = 16, 128, 512, 128, 128
NCORES = 8
QC = Q // NCORES  # 16 queries per core per batch
MASK_VALUE = -1000000.0


def _build_graph(fds):
    """fds: per-batch pruned key extents (even, <= K), baked into the graph."""
    from concourse import bacc, tile, mybir, masks

    f32 = mybir.dt.float32
    bf16 = mybir.dt.bfloat16
    AF = mybir.ActivationFunctionType

    nc = bacc.Bacc("TRN2", target_bir_lowering=False, num_devices=NCORES)

    qT = nc.dram_tensor("qT", [B, D, QC], f32, kind="ExternalInput").ap()
    kT = nc.dram_tensor("kT", [B, D, K], bf16, kind="ExternalInput").ap()
    vals = nc.dram_tensor("vals", [B, K, D], bf16, kind="ExternalInput").ap()
    mask = nc.dram_tensor("mask", [B, 1, K], f32, kind="ExternalInput").ap()
    Wq = nc.dram_tensor("Wq", [D, H], f32, kind="ExternalInput").ap()
    Wk = nc.dram_tensor("Wk", [D, H], bf16, kind="ExternalInput").ap()
    wv = nc.dram_tensor("wv", [H, 1], f32, kind="ExternalInput").ap()
    out = nc.dram_tensor("out", [B, QC, D], f32, kind="ExternalOutput").ap()

    with tile.TileContext(nc) as tc:
        with (
            tc.tile_pool(name="const", bufs=1) as const,
            tc.tile_pool(name="inp", bufs=3) as inp,
            tc.tile_pool(name="proj_ps", bufs=1, space="PSUM") as proj_ps,
            tc.tile_pool(name="proj_sb", bufs=3) as proj_sb,
            tc.tile_pool(name="g_pool", bufs=3) as g_pool,
            tc.tile_pool(name="t_pool", bufs=3) as t_pool,
            tc.tile_pool(name="sc_ps", bufs=2, space="PSUM") as sc_ps,
            tc.tile_pool(name="soft_sb", bufs=3) as soft_sb,
            tc.tile_pool(name="at_ps", bufs=2, space="PSUM") as at_ps,
            tc.tile_pool(name="at_sb", bufs=3) as at_sb,
            tc.tile_pool(name="av_ps", bufs=2, space="PSUM") as av_ps,
            tc.tile_pool(name="out_sb", bufs=3) as out_sb,
        ):
            wq_t = const.tile([D, H], f32)
            nc.sync.dma_start(wq_t[:], Wq[:])
            wk_t = const.tile([D, H], bf16)
            nc.sync.dma_start(wk_t[:], Wk[:])
            wv_f32 = const.tile([H, 1], f32)
            nc.sync.dma_start(wv_f32[:], wv[:])
            # wv_diag[:, j*QC+j] = w_v, else 0: matvec with the (H, QC)
            # slice j writes w_v . t into row j of the PSUM section tile
            # and zeros into the other QC-1 rows.
            wv_diag = const.tile([H, QC * QC], bf16)
            nc.vector.memset(wv_diag[:], 0.0)
            for j in range(QC):
                nc.vector.tensor_copy(
                    wv_diag[:, j * QC + j : j * QC + j + 1], wv_f32[:]
                )
            ones_row = const.tile([1, QC], bf16)
            nc.vector.memset(ones_row[:], 1.0)
            ident = const.tile([128, 128], bf16)
            masks.make_identity(nc, ident[:])

            for b in range(B):
                fd = fds[b]
                nkt = (fd + 127) // 128  # 128-key tiles touched

                qT_sb = inp.tile([D, QC], f32, tag="qT_sb")
                nc.sync.dma_start(qT_sb[:], qT[b])
                kT_sb = inp.tile([D, fd], bf16, tag="kT_sb")
                nc.sync.dma_start(kT_sb[:], kT[b, :, :fd])
                vals_sb = inp.tile([128, nkt * D], bf16, tag="vals_sb")
                for kt in range(nkt):
                    nc.sync.dma_start(
                        vals_sb[:, kt * D : (kt + 1) * D],
                        vals[b, kt * 128 : (kt + 1) * 128, :],
                    )
                mask_sb = inp.tile([1, fd], f32, tag="mask_sb")
                nc.sync.dma_start(mask_sb[:], mask[b, :, :fd])
                mask_bf = inp.tile([1, fd], bf16, tag="mask_bf")
                nc.vector.tensor_copy(mask_bf[:], mask_sb[:])

                qpT_ps = proj_ps.tile([H, QC], f32, tag="qpT_ps")
                nc.tensor.matmul(qpT_ps[:], wq_t[:], qT_sb[:], start=True, stop=True)
                qpT = proj_sb.tile([H, QC], f32, tag="qpT")
                nc.vector.tensor_copy(qpT[:], qpT_ps[:])
                kpT_ps = proj_ps.tile([H, fd], f32, tag="kpT_ps")
                nc.tensor.matmul(kpT_ps[:], wk_t[:], kT_sb[:], start=True, stop=True)
                kpT = proj_sb.tile([H, fd], bf16, tag="kpT")
                nc.vector.tensor_copy(kpT[:], kpT_ps[:])

                sc = sc_ps.tile([QC, fd], f32, tag="sc")
                nc.tensor.matmul(
                    sc[:], ones_row[:], mask_bf[:],
                    start=True, stop=False, skip_group_check=True,
                )
                g = g_pool.tile([H, QC * fd], bf16, tag="g")
                for j in range(QC):
                    nc.vector.tensor_scalar_add(
                        g[:, j * fd : (j + 1) * fd], kpT[:], qpT[:, j : j + 1]
                    )
                tt = t_pool.tile([H, QC * fd], bf16, tag="tt")
                nc.scalar.activation(tt[:], g[:], AF.Tanh)
                for j in range(QC):
                    nc.tensor.matmul(
                        sc[:],
                        wv_diag[:, j * QC : (j + 1) * QC],
                        tt[:, j * fd : (j + 1) * fd],
                        start=False,
                        stop=(j == QC - 1),
                        skip_group_check=True,
                    )

                # softmax over the fd live keys (free axis)
                expo = soft_sb.tile([QC, nkt * 128], bf16, tag="expo")
                sumexp = soft_sb.tile([QC, 1], f32, tag="sumexp")
                nc.scalar.activation(
                    expo[:, :fd], sc[:], AF.Exp, accum_out=sumexp[:]
                )
                if fd < nkt * 128:
                    nc.vector.memset(expo[:, fd:], 0.0)
                rec = soft_sb.tile([QC, 1], f32, tag="rec")
                nc.vector.reciprocal(rec[:], sumexp[:])

                av = av_ps.tile([QC, D], f32, tag="av")
                for kt in range(nkt):
                    aT_ps = at_ps.tile([128, QC], bf16, tag="aT_ps")
                    nc.tensor.transpose(
                        aT_ps[:], expo[:, kt * 128 : (kt + 1) * 128], ident[:QC, :QC]
                    )
                    aT = at_sb.tile([128, QC], bf16, tag="aT")
                    nc.vector.tensor_copy(aT[:], aT_ps[:])
                    nc.tensor.matmul(
                        av[:],
                        aT[:],
                        vals_sb[:, kt * D : (kt + 1) * D],
                        start=(kt == 0),
                        stop=(kt == nkt - 1),
                    )
                ot = out_sb.tile([QC, D], f32, tag="ot")
                nc.vector.tensor_scalar_mul(ot[:], av[:], rec[:])
                nc.sync.dma_start(out[b], ot[:])

    nc.finalize()
    return nc


_NC_CACHE = {}


def _prep(queries, keys, values, valid_lens, W_q, W_k, w_v):
    """Returns (nc, in_maps) for the given full inputs."""
    import ml_dtypes

    bf = ml_dtypes.bfloat16
    queries = np.asarray(queries, dtype=np.float32)
    keys = np.asarray(keys, dtype=np.float32)
    values = np.asarray(values, dtype=np.float32)
    valid_lens = np.asarray(valid_lens).astype(np.int64)
    W_q = np.asarray(W_q, dtype=np.float32)
    W_k = np.asarray(W_k, dtype=np.float32)
    w_v = np.asarray(w_v, dtype=np.float32)

    fds = tuple(int(min(K, ((v + 1) // 2) * 2)) for v in valid_lens)

    if fds not in _NC_CACHE:
        _NC_CACHE[fds] = _build_graph(fds)
    nc = _NC_CACHE[fds]

    queriesT = np.ascontiguousarray(queries.transpose(0, 2, 1))  # (B, D, Q)
    keysT = np.ascontiguousarray(keys.transpose(0, 2, 1)).astype(bf)  # (B, D, K)
    vals_bf = values.astype(bf)
    mask_add = np.where(
        np.arange(K)[None, :] < valid_lens[:, None], 0.0, MASK_VALUE
    ).astype(np.float32)[:, None, :]  # (B, 1, K)
    wv2 = np.ascontiguousarray(w_v.reshape(H, 1))
    Wk_bf = W_k.astype(bf)

    in_maps = []
    for c in range(NCORES):
        in_maps.append(
            {
                "qT": np.ascontiguousarray(queriesT[:, :, c * QC : (c + 1) * QC]),
                "kT": keysT,
                "vals": vals_bf,
                "mask": mask_add,
                "Wq": W_q,
                "Wk": Wk_bf,
                "wv": wv2,
            }
        )
    return nc, in_maps


def _gather(res):
    out = np.empty((B, Q, D), dtype=np.float32)
    for c in range(NCORES):
        out[:, c * QC : (c + 1) * QC, :] = res.results[c]["out"]
    return out


def kernel(queries, keys, values, valid_lens, W_q, W_k, w_v):
    from concourse.bass_utils import run_bass_kernel_spmd

    nc, in_maps = _prep(queries, keys, values, valid_lens, W_q, W_k, w_v)
    res = run_bass_kernel_spmd(nc, in_maps, core_ids=list(range(NCORES)))
    return _gather(res)
